# revision 1
# baseline (speedup 1.0000x reference)
"""Trainium2 Bass kernel for nn_CustomAttention (B=8, S=1024, H=1024, NH=16).

Strategy: data-parallel over batch — one batch element per NeuronCore, no
collectives. Host does layout-only prep (transposes for sharding); all FLOPs
run on device.

Per-core dataflow (hsT = hidden_states[b].T, wXT = WX.T):
  QT[o,s] = sum_h wqT[h,o] * hsT[h,s]  (+bq via per-partition tensor_scalar)
  KT[o,s] likewise
  V[s,o]  = sum_h hsT[h,s] * wvT[h,o]  (+bv via K=1 ones-row matmul),
            stored per s-tile as V' [128, NH*65]: per head 64 cols of V plus
            a ones column (col 65) so the ctx matmul also produces the
            softmax denominator (sum over s of exp) in PSUM row 64.
  scoresT[s,l] per head = KT_h(stationary) . QT_h  -> PSUM [128, S]
  expT = Exp(scoresT * 1/sqrt(HD))  (ACT, PSUM->SBUF; no max-subtraction:
         scores ~ N(0,1) so exp is well-conditioned in fp32)
  ctx'T[d,l] (+denom row 64) = V'_h(stationary) . expT  accum over s-tiles
  ctx' drained to SBUF; per 4-head group: denom rows gathered (small DMAs),
  recip = exp(-ln(denom)) batched on ACT (same table set), row DMA'd to a
  partition-0 tile, partition_broadcast (GPSIMD), multiply (DVE), DMA out.

Matmuls run in float32r (full-rate; fp32 is 4 cycles/row). f32r rounds
operands to ~13 mantissa bits at the producer -> end-to-end scale-relative
error ~4e-4 vs the fp32 reference.
"""
import sys

sys.path.insert(0, "/opt/trn_rl_repo")

import numpy as np
from contextlib import ExitStack

from concourse import bacc, tile, mybir
from concourse.bass_utils import run_bass_kernel_spmd

F32 = mybir.dt.float32
F32R = mybir.dt.float32r
AF = mybir.ActivationFunctionType

P = 128
HD = 64
N_CORES = 8


def _chunks(total, size=512):
    out = []
    a = 0
    while a < total:
        out.append((a, min(a + size, total)))
        a += size
    return out


def build_program(S, H, NH, num_devices=N_CORES, reps=1):
    """One SPMD program; every core runs it on its own batch element.

    reps > 1 repeats the whole computation (timing harness only).
    """
    KT = H // P          # h-tiles (contraction tiles)
    NT = H // P          # o-tiles
    ST = S // P          # s-tiles
    HPT = P // HD        # heads per o-tile (2)
    assert NH * HD == H and HPT == 2
    SCALE = 1.0 / float(np.sqrt(HD))

    nc = bacc.Bacc(
        "TRN2", target_bir_lowering=False, debug=False, num_devices=num_devices
    )

    hsT = nc.dram_tensor("hsT", [H, S], F32, kind="ExternalInput")
    wqT = nc.dram_tensor("wqT", [H, H], F32, kind="ExternalInput")
    wkT = nc.dram_tensor("wkT", [H, H], F32, kind="ExternalInput")
    wvT = nc.dram_tensor("wvT", [H, H], F32, kind="ExternalInput")
    bqT = nc.dram_tensor("bqT", [P, NT], F32, kind="ExternalInput")
    bkT = nc.dram_tensor("bkT", [P, NT], F32, kind="ExternalInput")
    # host-provided constants so every matmul operand is DMA-produced --
    # walrus requires f32r matmul inputs to come from f32r-typed producers
    bv_row = nc.dram_tensor("bv_row", [1, H], F32, kind="ExternalInput")
    ones_row = nc.dram_tensor("ones_row", [1, P], F32, kind="ExternalInput")
    onescol = nc.dram_tensor("onescol", [P, NH], F32, kind="ExternalInput")
    outT = nc.dram_tensor("outT", [H, S], F32, kind="ExternalOutput")

    with tile.TileContext(nc) as tc, ExitStack() as ctx:
        consts = ctx.enter_context(tc.tile_pool(name="consts", bufs=1))
        hstp = ctx.enter_context(tc.tile_pool(name="hstp", bufs=KT))
        wstr = ctx.enter_context(tc.tile_pool(name="wstr", bufs=2))
        qtp = ctx.enter_context(tc.tile_pool(name="qtp", bufs=min(4, NT)))
        ktp = ctx.enter_context(tc.tile_pool(name="ktp", bufs=min(4, NT)))
        vvp = ctx.enter_context(tc.tile_pool(name="vvp", bufs=ST))
        big = ctx.enter_context(tc.tile_pool(name="big", bufs=2, space="PSUM"))
        cxp = ctx.enter_context(tc.tile_pool(name="cxp", bufs=2, space="PSUM"))

        # ---- constants ----
        bqT_sb = consts.tile([P, NT], F32, tag="bqT")
        bkT_sb = consts.tile([P, NT], F32, tag="bkT")
        bv_sb = consts.tile([1, H], F32R, tag="bv")
        ones_sb = consts.tile([1, P], F32R, tag="ones")
        nc.sync.dma_start(out=bqT_sb[:], in_=bqT[:])
        nc.sync.dma_start(out=bkT_sb[:], in_=bkT[:])
        nc.sync.dma_start(out=bv_sb[:], in_=bv_row[:].bitcast(F32R))
        nc.sync.dma_start(out=ones_sb[:], in_=ones_row[:].bitcast(F32R))

        pools2 = {}

        for rep in range(reps):
            # ---- load hsT ----
            ht = []
            for k in range(KT):
                t_ = hstp.tile([P, S], F32R, tag="ht", name=f"ht{rep}_{k}")
                nc.sync.dma_start(
                    out=t_[:], in_=hsT[k * P : (k + 1) * P, :].bitcast(F32R)
                )
                ht.append(t_)

            # ---- V' production (wvT streamed through a scoped pool) ----
            vv = []
            assert S == H, "big PSUM pool assumes S == H tile sizes"
            with tc.tile_pool(name=f"wvp{rep}", bufs=KT) as wvp:
                wv = []
                for k in range(KT):
                    t_ = wvp.tile([P, H], F32R, tag="wv", name=f"wv{rep}_{k}")
                    nc.sync.dma_start(
                        out=t_[:], in_=wvT[k * P : (k + 1) * P, :].bitcast(F32R)
                    )
                    wv.append(t_)

                for m in range(ST):
                    ps = big.tile([P, H], F32, tag="big", name=f"vps{rep}_{m}")
                    for (a, b) in _chunks(H):
                        nc.tensor.matmul(
                            ps[:, a:b], ones_sb[:], bv_sb[:, a:b],
                            start=True, stop=False,
                        )
                    for k in range(KT):
                        lhs = ht[k][:, m * P : (m + 1) * P]
                        for (a, b) in _chunks(H):
                            nc.tensor.matmul(
                                ps[:, a:b], lhs, wv[k][:, a:b],
                                start=False, stop=(k == KT - 1),
                            )
                    vt = vvp.tile(
                        [P, NH * 65], F32R, tag="vv", name=f"vv{rep}_{m}"
                    )
                    vview = vt[:].rearrange("p (h e) -> p h e", e=65)
                    nc.vector.tensor_copy(
                        vview[:, :, 0:64],
                        ps[:].rearrange("p (h d) -> p h d", d=HD),
                    )
                    nc.sync.dma_start(
                        out=vview[:, :, 64:65], in_=onescol[:].bitcast(F32R)
                    )
                    vv.append(vt)

            if not pools2:
                pools2["exp_pool"] = ctx.enter_context(
                    tc.tile_pool(name="exp_pool", bufs=5)
                )
                pools2["cup"] = ctx.enter_context(tc.tile_pool(name="cup", bufs=8))
                pools2["denp"] = ctx.enter_context(tc.tile_pool(name="denp", bufs=2))
                pools2["bsp"] = ctx.enter_context(tc.tile_pool(name="bsp", bufs=2))
                pools2["bcp"] = ctx.enter_context(tc.tile_pool(name="bcp", bufs=2))
                pools2["outp"] = ctx.enter_context(tc.tile_pool(name="outp", bufs=3))
            exp_pool = pools2["exp_pool"]
            cup = pools2["cup"]
            denp = pools2["denp"]
            bsp = pools2["bsp"]
            bcp = pools2["bcp"]
            outp = pools2["outp"]

            # ---- per o-tile: QT/KT projection then attention for its heads --
            group_cu = []  # drained ctx' tiles of the current 4-head group
            for t in range(NT):
                proj_out = []
                for (wT, bias_sb, pool, tag) in (
                    (wqT, bqT_sb, qtp, "qt"),
                    (wkT, bkT_sb, ktp, "kt"),
                ):
                    wtile = wstr.tile(
                        [P, KT, P], F32R, tag="wstr", name=f"w{tag}{rep}_{t}"
                    )
                    nc.sync.dma_start(
                        out=wtile[:],
                        in_=wT[:, t * P : (t + 1) * P]
                        .rearrange("(k p) c -> p k c", p=P)
                        .bitcast(F32R),
                    )
                    ps = big.tile([P, S], F32, tag="big", name=f"pps{rep}_{t}{tag}")
                    for k in range(KT):
                        for (a, b) in _chunks(S):
                            nc.tensor.matmul(
                                ps[:, a:b], wtile[:, k, :], ht[k][:, a:b],
                                start=(k == 0), stop=(k == KT - 1),
                            )
                    ot = pool.tile([P, S], F32R, tag=tag, name=f"{tag}{rep}_{t}")
                    nc.vector.tensor_scalar_add(
                        ot[:], ps[:], bias_sb[:, t : t + 1]
                    )
                    proj_out.append(ot)
                qt_t, kt_t = proj_out

                # heads 2t (rows 0:64) and 2t+1 (rows 64:128)
                cx = [
                    cxp.tile([65, S], F32, tag="cx", name=f"cx{rep}_{t}_{i}")
                    for i in range(HPT)
                ]
                for j in range(ST):
                    for hh in range(HPT):
                        r0, r1 = hh * HD, (hh + 1) * HD
                        sc = big.tile(
                            [P, S], F32, tag="big", name=f"sc{rep}_{t}_{j}_{hh}"
                        )
                        for (a, b) in _chunks(S):
                            nc.tensor.matmul(
                                sc[:, a:b],
                                kt_t[r0:r1, j * P : (j + 1) * P],
                                qt_t[r0:r1, a:b],
                                start=True, stop=True,
                                tile_position=(r0, 0),
                            )
                        ex = exp_pool.tile(
                            [P, S], F32R, tag="ex", name=f"ex{rep}_{t}_{j}_{hh}"
                        )
                        nc.scalar.activation(ex[:], sc[:], AF.Exp, scale=SCALE)
                        h = HPT * t + hh
                        lhs = vv[j][:, h * 65 : (h + 1) * 65]
                        for (a, b) in _chunks(S):
                            nc.tensor.matmul(
                                cx[hh][0:65, a:b], lhs, ex[:, a:b],
                                start=(j == 0), stop=(j == ST - 1),
                            )

                # drain ctx' to SBUF (frees PSUM quickly)
                for hh in range(HPT):
                    cu = cup.tile(
                        [65, S], F32, tag="cu", name=f"cu{rep}_{t}_{hh}"
                    )
                    nc.vector.tensor_copy(cu[:], cx[hh][:])
                    group_cu.append((HPT * t + hh, cu))

                # normalization for each 4-head group (2 o-tiles)
                if t % 2 == 1:
                    g = len(group_cu)
                    den = denp.tile([g, S], F32, tag="den", name=f"den{rep}_{t}")
                    for i, (h, cu) in enumerate(group_cu):
                        nc.sync.dma_start(
                            out=den[i : i + 1, :], in_=cu[64:65, :]
                        )
                    nc.scalar.activation(den[:], den[:], AF.Ln)
                    nc.scalar.activation(den[:], den[:], AF.Exp, scale=-1.0)
                    for i, (h, cu) in enumerate(group_cu):
                        bsrc = bsp.tile([1, S], F32, tag="bsrc", name=f"bs{rep}_{h}")
                        nc.sync.dma_start(out=bsrc[:], in_=den[i : i + 1, :])
                        bc = bcp.tile([HD, S], F32, tag="bc", name=f"bc{rep}_{h}")
                        nc.gpsimd.partition_broadcast(bc[:], bsrc[:])
                        ou = outp.tile([HD, S], F32, tag="ou", name=f"ou{rep}_{h}")
                        nc.vector.tensor_mul(ou[:], cu[0:64, :], bc[:])
                        nc.sync.dma_start(
                            out=outT[h * HD : (h + 1) * HD, :], in_=ou[:]
                        )
                    group_cu = []

    nc.compile()
    return nc


_CACHE = {}


def _get_program(S, H, NH, num_devices):
    key = (S, H, NH, num_devices)
    if key not in _CACHE:
        _CACHE[key] = build_program(S, H, NH, num_devices)
    return _CACHE[key]


def make_in_maps(hidden_states, Wq, bq, Wk, bk, Wv, bv):
    B, S, H = hidden_states.shape
    NH = H // HD
    NT = H // P
    wqT = np.ascontiguousarray(Wq.T.astype(np.float32))
    wkT = np.ascontiguousarray(Wk.T.astype(np.float32))
    wvT = np.ascontiguousarray(Wv.T.astype(np.float32))
    bqT = np.ascontiguousarray(bq.reshape(NT, P).T.astype(np.float32))
    bkT = np.ascontiguousarray(bk.reshape(NT, P).T.astype(np.float32))
    bv_row = bv.astype(np.float32).reshape(1, H)
    ones_row = np.ones((1, P), np.float32)
    ones_col = np.ones((P, NH), np.float32)
    in_maps = []
    for b in range(B):
        in_maps.append(
            {
                "hsT": np.ascontiguousarray(hidden_states[b].T.astype(np.float32)),
                "wqT": wqT,
                "wkT": wkT,
                "wvT": wvT,
                "bqT": bqT,
                "bkT": bkT,
                "bv_row": bv_row,
                "ones_row": ones_row,
                "onescol": ones_col,
            }
        )
    return in_maps


def kernel(hidden_states, Wq, bq, Wk, bk, Wv, bv):
    hidden_states = np.asarray(hidden_states, dtype=np.float32)
    Wq = np.asarray(Wq, dtype=np.float32)
    bq = np.asarray(bq, dtype=np.float32)
    Wk = np.asarray(Wk, dtype=np.float32)
    bk = np.asarray(bk, dtype=np.float32)
    Wv = np.asarray(Wv, dtype=np.float32)
    bv = np.asarray(bv, dtype=np.float32)

    B, S, H = hidden_states.shape
    NH = H // HD
    assert B == N_CORES, "one batch element per core"

    nc = _get_program(S, H, NH, N_CORES)
    in_maps = make_in_maps(hidden_states, Wq, bq, Wk, bk, Wv, bv)
    res = run_bass_kernel_spmd(nc, in_maps, core_ids=list(range(N_CORES)))
    out = np.empty((B, S, H), np.float32)
    for b in range(B):
        out[b] = res.results[b]["outT"].T
    return out


if __name__ == "__main__":
    build_program(1024, 1024, 16)
    print("build ok")



# revision 3
# speedup vs baseline: 1.4992x; 1.4992x over previous
"""Trainium2 Bass kernel for nn_CustomAttention (B=8, S=1024, H=1024, NH=16).

Strategy: data-parallel over batch — one batch element per NeuronCore, no
collectives. Host does layout-only prep (transposes + f16 casts); all FLOPs
run on device.

Per-core dataflow (hsT = hidden_states[b].T, wXT = WX.T, all f16):
  QT[o,s] = sum_h wqT[h,o] hsT[h,s]; drain adds bq and folds 1/sqrt(HD)
            (DVE tensor_scalar add+mult, f16 out).  KT likewise (add only).
  V[s,o]  = sum_h hsT[h,s] wvT[h,o]; drain adds bv (Pool tensor_tensor with a
            partition-broadcast bias tile) into V' tiles [128, NH*65] laid out
            per head as 64 value cols + a ones col, so the ctx matmul also
            emits the softmax denominator.
  scoresT[kv,l] per (head, kv-tile) = K_h(stationary, 64 rows) . Q_h -> PSUM.
  exp on ACT only (PSUM->SBUF f16); logits already scaled at the Q drain, so
  no max-subtraction (logits ~ N(0,1), exp well-conditioned; table never
  swaps away from Exp).
  ctx^T per (head, l-block): stationary = exp[128 kv, 128 l] slice, moving =
  V'_h [128, 65] -> out [128 l, 65] PSUM, accumulated over kv tiles; col 64 is
  the denominator per l-partition. Eight l-block regions pack 128-col-aligned
  into one 2-bank PSUM tile (same-bank groups sequential: l-outer, kv-inner).
  Normalize: DVE gathers the denom cols [128,8], reciprocal, then one
  tensor_tensor multiply with a stride-0 broadcast writes the final [128,8,64]
  f32 staging tile, DMA'd straight to out[S,H].

Everything is f16 (fp8/DoubleRow blows the 2e-2 absmax gate: fp8 QK proj
alone is 7.5e-2). f16 keeps PE at 1 cycle/moving-col and total error ~1e-3.
"""
import sys

sys.path.insert(0, "/opt/trn_rl_repo")

import numpy as np
from contextlib import ExitStack

from concourse import bacc, tile, mybir
from concourse.bass_utils import run_bass_kernel_spmd

F32 = mybir.dt.float32
F16 = mybir.dt.float16
AF = mybir.ActivationFunctionType
ADD = mybir.AluOpType.add
MULT = mybir.AluOpType.mult

P = 128
HD = 64
N_CORES = 8


def build_program(S, H, NH, num_devices=N_CORES):
    KT = H // P           # contraction tiles (8)
    NT = H // P           # o tiles (8)
    ST = S // P           # s / kv / l tiles (8)
    HPT = P // HD         # heads per o-tile (2)
    assert NH * HD == H and HPT == 2 and S == H
    SCALE = 1.0 / float(np.sqrt(HD))

    nc = bacc.Bacc(
        "TRN2", target_bir_lowering=False, debug=False, num_devices=num_devices
    )

    hsT = nc.dram_tensor("hsT", [H, S], F16, kind="ExternalInput")
    wqT = nc.dram_tensor("wqT", [H, H], F16, kind="ExternalInput")
    wkT = nc.dram_tensor("wkT", [H, H], F16, kind="ExternalInput")
    wvT = nc.dram_tensor("wvT", [H, H], F16, kind="ExternalInput")
    bqT = nc.dram_tensor("bqT", [P, NT], F32, kind="ExternalInput")
    bkT = nc.dram_tensor("bkT", [P, NT], F32, kind="ExternalInput")
    bv_row = nc.dram_tensor("bv_row", [H], F32, kind="ExternalInput")
    out = nc.dram_tensor("out", [S, H], F32, kind="ExternalOutput")
    # [S, H] rows l = lb*128 + p  ->  [p, lb, h-cols]
    out_v = out[:].rearrange("(lb p) c -> p lb c", p=P)

    with tile.TileContext(nc) as tc, ExitStack() as ctx:
        consts = ctx.enter_context(tc.tile_pool(name="consts", bufs=1))
        hstp = ctx.enter_context(tc.tile_pool(name="hstp", bufs=KT))
        wvp = ctx.enter_context(tc.tile_pool(name="wvp", bufs=KT))
        wstr = ctx.enter_context(tc.tile_pool(name="wstr", bufs=4))
        qtp = ctx.enter_context(tc.tile_pool(name="qtp", bufs=NT))
        ktp = ctx.enter_context(tc.tile_pool(name="ktp", bufs=NT))
        vvp = ctx.enter_context(tc.tile_pool(name="vvp", bufs=ST))
        exp_pool = ctx.enter_context(tc.tile_pool(name="exp_pool", bufs=3 * ST))
        osbp = ctx.enter_context(tc.tile_pool(name="osbp", bufs=2))
        denp = ctx.enter_context(tc.tile_pool(name="denp", bufs=2))
        recp = ctx.enter_context(tc.tile_pool(name="recp", bufs=2))
        # PSUM: scores 2x2 banks + ctx/V 2 + proj 2 = 8 banks exactly
        big = ctx.enter_context(tc.tile_pool(name="big", bufs=2, space="PSUM"))
        cxv = ctx.enter_context(tc.tile_pool(name="cxv", bufs=1, space="PSUM"))
        prp = ctx.enter_context(tc.tile_pool(name="prp", bufs=1, space="PSUM"))

        # ---- weight DMAs for o-tile 0 first so proj(0) starts immediately --
        def load_w(wT, t, tag):
            wt = wstr.tile([P, KT, P], F16, tag="wstr", name=f"w{tag}{t}")
            nc.sync.dma_start(
                out=wt[:],
                in_=wT[:, t * P:(t + 1) * P].rearrange("(k p) c -> p k c", p=P),
            )
            return wt

        wq0 = load_w(wqT, 0, "q")
        wk0 = load_w(wkT, 0, "k")

        bqT_sb = consts.tile([P, NT], F32, tag="bqT")
        bkT_sb = consts.tile([P, NT], F32, tag="bkT")
        nc.sync.dma_start(out=bqT_sb[:], in_=bqT[:])
        nc.sync.dma_start(out=bkT_sb[:], in_=bkT[:])

        ht = []
        for k in range(KT):
            t_ = hstp.tile([P, S], F16, tag="ht", name=f"ht{k}")
            nc.sync.dma_start(out=t_[:], in_=hsT[k * P:(k + 1) * P, :])
            ht.append(t_)

        wv = []
        for k in range(KT):
            t_ = wvp.tile([P, H], F16, tag="wv", name=f"wv{k}")
            nc.sync.dma_start(out=t_[:], in_=wvT[k * P:(k + 1) * P, :])
            wv.append(t_)

        bv_sb = consts.tile([P, H], F32, tag="bvb")
        nc.sync.dma_start(out=bv_sb[:], in_=bv_row[:].partition_broadcast(P))

        # V' tiles with ones columns pre-set (col 64 of each head's 65)
        vv = []
        for m in range(ST):
            vt = vvp.tile([P, NH * 65], F16, tag="vv", name=f"vv{m}")
            vview = vt[:].rearrange("p (h e) -> p h e", e=65)
            nc.vector.memset(vview[:, :, 64:65], 1.0)
            vv.append(vt)

        qt = [None] * NT
        kt = [None] * NT
        wtiles = {0: (wq0, wk0)}

        def proj(t):
            wqt, wkt = wtiles.pop(t)
            for (wt, bias, pool, tag, do_scale) in (
                (wqt, bqT_sb, qtp, "q", True),
                (wkt, bkT_sb, ktp, "k", False),
            ):
                ps = prp.tile([P, S], F32, tag="pr", name=f"pr{tag}{t}")
                for k in range(KT):
                    for c in range(2):
                        nc.tensor.matmul(
                            ps[:, c * 512:(c + 1) * 512],
                            wt[:, k, :],
                            ht[k][:, c * 512:(c + 1) * 512],
                            start=(k == 0), stop=(k == KT - 1),
                        )
                ot = pool.tile([P, S], F16, tag=tag, name=f"{tag}t{t}")
                if do_scale:
                    nc.vector.tensor_scalar(
                        ot[:], ps[:], bias[:, t:t + 1], SCALE, ADD, MULT
                    )
                else:
                    nc.vector.tensor_scalar_add(ot[:], ps[:], bias[:, t:t + 1])
                if tag == "q":
                    qt[t] = ot
                else:
                    kt[t] = ot

        def vprod(m):
            ps = cxv.tile([P, H], F32, tag="cxv", name=f"vps{m}")
            for k in range(KT):
                for c in range(2):
                    nc.tensor.matmul(
                        ps[:, c * 512:(c + 1) * 512],
                        ht[k][:, m * P:(m + 1) * P],
                        wv[k][:, c * 512:(c + 1) * 512],
                        start=(k == 0), stop=(k == KT - 1),
                    )
            vview = vv[m][:].rearrange("p (h e) -> p h e", e=65)
            nc.vector.tensor_tensor(
                vview[:, :, 0:64],
                ps[:].rearrange("p (h d) -> p h d", d=HD),
                bv_sb[:].rearrange("p (h d) -> p h d", d=HD),
                ADD,
            )

        ex_tiles = {}

        def scores_exp(h):
            t, hh = divmod(h, HPT)
            r0, r1 = hh * HD, (hh + 1) * HD
            exs = []
            for j in range(ST):
                sc = big.tile([P, S], F32, tag="big", name=f"sc{h}_{j}")
                for c in range(2):
                    nc.tensor.matmul(
                        sc[:, c * 512:(c + 1) * 512],
                        kt[t][r0:r1, j * P:(j + 1) * P],
                        qt[t][r0:r1, c * 512:(c + 1) * 512],
                        start=True, stop=True,
                        tile_position=(r0, 0),
                    )
                ex = exp_pool.tile([P, S], F16, tag="ex", name=f"ex{h}_{j}")
                nc.scalar.activation(ex[:], sc[:], AF.Exp)
                exs.append(ex)
            ex_tiles[h] = exs

        def ctx_head(h):
            exs = ex_tiles.pop(h)
            ps = cxv.tile([P, ST * P], F32, tag="cxv", name=f"cx{h}")
            for lb in range(ST):
                for j in range(ST):
                    nc.tensor.matmul(
                        ps[:, lb * P:lb * P + 65],
                        exs[j][:, lb * P:(lb + 1) * P],
                        vv[j][:, h * 65:(h + 1) * 65],
                        start=(j == 0), stop=(j == ST - 1),
                    )
            ps3 = ps[:].rearrange("p (lb c) -> p lb c", c=P)
            den = denp.tile([P, ST], F32, tag="den", name=f"den{h}")
            nc.vector.tensor_copy(den[:], ps3[:, :, 64:65].rearrange("p a b -> p (a b)"))
            rec = recp.tile([P, ST], F32, tag="rec", name=f"rec{h}")
            nc.vector.reciprocal(rec[:], den[:])
            osb = osbp.tile([P, ST, HD], F32, tag="osb", name=f"osb{h}")
            nc.vector.tensor_tensor(
                osb[:], ps3[:, :, 0:64], rec[:].broadcast_to([P, ST, HD]), MULT
            )
            nc.sync.dma_start(
                out=out_v[:, :, h * HD:(h + 1) * HD], in_=osb[:]
            )

        # ---- PE issue order: keep PE saturated, ACT fed from head 0 on ----
        proj(0)
        vprod(0); vprod(1)
        scores_exp(0)
        vprod(2); vprod(3)
        scores_exp(1)
        vprod(4); vprod(5)
        wtiles[1] = (load_w(wqT, 1, "q"), load_w(wkT, 1, "k"))
        proj(1)
        vprod(6); vprod(7)
        ctx_head(0)
        scores_exp(2)
        ctx_head(1)
        wtiles[2] = (load_w(wqT, 2, "q"), load_w(wkT, 2, "k"))
        proj(2)
        for h in range(3, 2 * NT):
            scores_exp(h)
            ctx_head(h - 1)
            t_next = h // 2 + 2
            if h % 2 == 1 and t_next < NT:
                wtiles[t_next] = (
                    load_w(wqT, t_next, "q"), load_w(wkT, t_next, "k")
                )
                proj(t_next)
        ctx_head(2 * NT - 1)

    nc.compile()
    return nc


_CACHE = {}


def _get_program(S, H, NH, num_devices):
    key = (S, H, NH, num_devices)
    if key not in _CACHE:
        _CACHE[key] = build_program(S, H, NH, num_devices)
    return _CACHE[key]


def make_in_maps(hidden_states, Wq, bq, Wk, bk, Wv, bv):
    B, S, H = hidden_states.shape
    NT = H // P
    wqT = np.ascontiguousarray(Wq.T).astype(np.float16)
    wkT = np.ascontiguousarray(Wk.T).astype(np.float16)
    wvT = np.ascontiguousarray(Wv.T).astype(np.float16)
    bqT = np.ascontiguousarray(bq.reshape(NT, P).T.astype(np.float32))
    bkT = np.ascontiguousarray(bk.reshape(NT, P).T.astype(np.float32))
    bvr = bv.astype(np.float32)
    in_maps = []
    for b in range(B):
        in_maps.append(
            {
                "hsT": np.ascontiguousarray(hidden_states[b].T).astype(np.float16),
                "wqT": wqT,
                "wkT": wkT,
                "wvT": wvT,
                "bqT": bqT,
                "bkT": bkT,
                "bv_row": bvr,
            }
        )
    return in_maps


def kernel(hidden_states, Wq, bq, Wk, bk, Wv, bv):
    hidden_states = np.asarray(hidden_states, dtype=np.float32)
    Wq = np.asarray(Wq, dtype=np.float32)
    bq = np.asarray(bq, dtype=np.float32)
    Wk = np.asarray(Wk, dtype=np.float32)
    bk = np.asarray(bk, dtype=np.float32)
    Wv = np.asarray(Wv, dtype=np.float32)
    bv = np.asarray(bv, dtype=np.float32)

    B, S, H = hidden_states.shape
    NH = H // HD
    assert B == N_CORES, "one batch element per core"

    nc = _get_program(S, H, NH, N_CORES)
    in_maps = make_in_maps(hidden_states, Wq, bq, Wk, bk, Wv, bv)
    res = run_bass_kernel_spmd(nc, in_maps, core_ids=list(range(N_CORES)))
    out = np.empty((B, S, H), np.float32)
    for b in range(B):
        out[b] = res.results[b]["out"]
    return out


if __name__ == "__main__":
    build_program(1024, 1024, 16)
    print("build ok")
